# revision 23
# baseline (speedup 1.0000x reference)
"""Trainium2 Bass kernel for batched scaled-dot-product attention.

Problem (reference math in fp32):
    q = queries @ Wq + bq          [B=4, N=4096, E=64]   (D_MODEL=768)
    k = keys    @ Wk + bk
    v = values  @ Wv + bv
    out = softmax(q k^T / sqrt(E)) @ v                    [B, N, 64]

Sharding: 8 cores, data-parallel over batch x query-half.  Core c handles
batch b=c//2, query rows [h*2048, (h+1)*2048) with h=c%2; it loads the full
keys/values for its batch (softmax needs every key).

v2 design (vs the fp32r v1 baseline at ~176us):
  * Everything on the input path is bf16 (host-cast): x DMA bytes halve to
    ~12MB/core and every matmul runs at 1 cycle/row at any PE p-state.
    Verified numerically: end-to-end rel err ~5.5e-3 vs the 2e-2 gate.
  * No q/k row-doubling: bf16 matmuls don't need a 128-deep contraction to
    hit full rate (the moving-row stream is the limit either way).
  * The 1/sqrt(E) scale is folded into Wq/bq on the host.
  * v is projected straight into natural [seq,64] layout ("va-direct"):
    per 128-row tile, 6 matmuls with the x_v^T chunk as the stationary
    operand.  No PE/DMA transposes anywhere in the main pipeline.  Two ones
    columns are appended (va width 66) so attention row-sums fall out of
    the av matmul; normalization happens on the HOST after gather.
  * Attention in S^T layout.  Query groups 0-2 stream inline with the k/v
    projection (per k-tile: 3 S matmuls, a paired exp on groups 0+1 plus a
    single exp on group 2, 3 av accumulations).  Group 3 runs as a second
    pass over resident kT/qT/va with kt-paired exps.  This 3+1 split is
    what fits 8 PSUM banks: S pool 2x[128,2,512] (4) + oT 3x[66,512] (3) +
    projection accumulator (1).
  * exp is the ACT-engine floor (~55us of pure column throughput); pairing
    two 512-col scores tiles per activation instruction halves the ~143ns
    per-instruction overhead.  ACT does nothing but exp.
  * Output is written as oT [66, 2048] fp32 (64 value rows + rowsum row);
    the host does out = (oT[:64]/oT[64]).T -- no device epilogue transpose.
"""

import numpy as np
import ml_dtypes

B, N, D, E = 4, 4096, 768, 64
NCORES = 8
HALF = N // 2          # query rows per core
CH = D // 128          # 6 feature chunks of the contraction dim
GROUP = 512            # query columns per group
QG = HALF // GROUP     # 4 query groups per core
KT = N // 128          # 32 key tiles
KG = N // GROUP        # 8 k/v projection groups
MA = E + 2             # va width: 64 values + 2 ones columns (rowsum)
SCALE = 0.125          # 1/sqrt(E), folded into Wq/bq on the host

_CACHE = {}


def _build():
    from contextlib import ExitStack

    import concourse.mybir as mybir
    import concourse.tile as tile
    from concourse import bacc

    from concourse.masks import make_identity

    f32 = mybir.dt.float32
    f32r = mybir.dt.float32r
    bf16 = mybir.dt.bfloat16
    EXP = mybir.ActivationFunctionType.Exp

    nc = bacc.Bacc(trn_type="TRN2")
    x_q = nc.dram_tensor("x_q", [D, HALF], bf16, kind="ExternalInput")
    x_k = nc.dram_tensor("x_k", [D, N], bf16, kind="ExternalInput")
    x_v = nc.dram_tensor("x_v", [D, N], bf16, kind="ExternalInput")
    w_q = nc.dram_tensor("w_q", [128, CH, E], bf16, kind="ExternalInput")
    w_k = nc.dram_tensor("w_k", [128, CH, E], bf16, kind="ExternalInput")
    w_v = nc.dram_tensor("w_v", [128, CH, E], bf16, kind="ExternalInput")
    b_q = nc.dram_tensor("b_q", [E, 1], f32, kind="ExternalInput")
    b_k = nc.dram_tensor("b_k", [E, 1], f32, kind="ExternalInput")
    b_v = nc.dram_tensor("b_v", [E, 1], f32, kind="ExternalInput")
    out = nc.dram_tensor("out", [MA, HALF], f32, kind="ExternalOutput")
    import os
    debug = bool(os.environ.get("KERNEL_DEBUG_DUMP"))
    if debug:
        dbg_qT = nc.dram_tensor("dbg_qT", [E, HALF], bf16, kind="ExternalOutput")
        dbg_kT = nc.dram_tensor("dbg_kT", [E, N], bf16, kind="ExternalOutput")
        dbg_va = nc.dram_tensor("dbg_va", [128, KT, MA], bf16, kind="ExternalOutput")

    with tile.TileContext(nc) as tc, ExitStack() as ctx:
        singles = ctx.enter_context(tc.tile_pool(name="singles", bufs=1))
        wq_sb = singles.tile([128, CH, E], bf16)
        wk_sb = singles.tile([128, CH, E], bf16)
        wv_sb = singles.tile([128, CH, E], bf16)
        bq_sb = singles.tile([E, 1], f32)
        bk_sb = singles.tile([E, 1], f32)
        bv_sb = singles.tile([E, 1], f32)
        for dst, src in ((wq_sb, w_q), (wk_sb, w_k), (wv_sb, w_v),
                         (bq_sb, b_q), (bk_sb, b_k), (bv_sb, b_v)):
            nc.sync.dma_start(out=dst, in_=src[:])

        qT = singles.tile([E, HALF], bf16)      # q^T/8 (+bias)
        kT = singles.tile([E, N], bf16)         # k^T
        vT = singles.tile([E, N], f32r)         # v^T (+bias)
        va = singles.tile([128, KT, MA], bf16)  # v natural + two ones columns
        nc.vector.memset(va[:, :, E:MA], 1.0)
        identf = singles.tile([128, 128], f32)
        make_identity(nc, identf)
        ident = singles.tile([128, 128], f32r)
        nc.vector.tensor_copy(ident, identf)

        xpool = ctx.enter_context(tc.tile_pool(name="xT", bufs=8))
        pt01 = ctx.enter_context(tc.tile_pool(name="pt01", bufs=3))
        pt2 = ctx.enter_context(tc.tile_pool(name="pt2", bufs=3))
        eppool = ctx.enter_context(tc.tile_pool(name="ep", bufs=2))
        spool = ctx.enter_context(tc.tile_pool(name="s", bufs=2, space="PSUM"))
        opool = ctx.enter_context(tc.tile_pool(name="o", bufs=3, space="PSUM"))
        pjpool = ctx.enter_context(tc.tile_pool(name="pj", bufs=1, space="PSUM"))

        def load_x(x_dr, g):
            xt = xpool.tile([128, CH, GROUP], bf16, tag="xT", name="xt")
            nc.sync.dma_start(
                out=xt,
                in_=x_dr[:, g * GROUP:(g + 1) * GROUP].rearrange(
                    "(c p) s -> p c s", p=128))
            return xt

        def proj_qk(xt, w_sb, b_sb, dst, g):
            ps = pjpool.tile([128, GROUP], f32, tag="pj", name="ps")
            for c in range(CH):
                nc.tensor.matmul(
                    ps[:E], lhsT=w_sb[:, c, :], rhs=xt[:, c, :],
                    start=(c == 0), stop=(c == CH - 1))
            nc.vector.tensor_scalar_add(
                dst[:, g * GROUP:(g + 1) * GROUP], ps[:E], b_sb)

        def proj_va_mm(xt, g8):
            # Project v exactly like k: 512-wide moves amortize LDWEIGHTS,
            # bias is per-partition in the vT layout.
            proj_qk(xt, wv_sb, bv_sb, vT, g8)

        def va_tp(kt):
            # Flip one 128-seq tile of vT into natural layout on the PE.
            tp = pjpool.tile([128, E], f32r, tag="pj", name="tp")
            nc.tensor.transpose(
                tp, vT[:, kt * 128:(kt + 1) * 128], ident[:E, :E])
            nc.vector.tensor_copy(va[:, kt, :E], tp)

        def s_step(kt, g, out_ap):
            nc.tensor.matmul(
                out_ap,
                lhsT=kT[:, kt * 128:(kt + 1) * 128],
                rhs=qT[:, g * GROUP:(g + 1) * GROUP],
                start=True, stop=True, skip_group_check=True)

        def av_step(oT_g, kt, pt_ap, first, last):
            nc.tensor.matmul(
                oT_g, lhsT=va[:, kt, :], rhs=pt_ap,
                start=first, stop=last, skip_group_check=True)

        def epilogue(g, oT_g):
            o_sb = eppool.tile([MA, GROUP], f32, tag="ep", name="o_sb")
            nc.vector.tensor_copy(o_sb, oT_g)
            nc.sync.dma_start(out=out[:, g * GROUP:(g + 1) * GROUP], in_=o_sb)

        # ---- prologue: q0 + k/v group 0 first to minimize attention lag ----
        xq0 = load_x(x_q, 0)
        xk_cur = load_x(x_k, 0)
        xv_cur = load_x(x_v, 0)
        xq1 = load_x(x_q, 1)
        xq2 = load_x(x_q, 2)
        proj_qk(xq0, wq_sb, bq_sb, qT, 0)

        # ---- pass 1: k/v stream + attention for groups 0,1,2 ----
        # Software-pipelined: per kt we issue S matmuls + exps for kt but the
        # av accumulations for kt-1, so the in-order PE never waits on the
        # exp it just requested.  Attention kt blocks for group g8-1 are
        # issued around g8's projections so the PE has work while the DVE
        # drains the single projection PSUM bank.
        oT = [opool.tile([MA, GROUP], f32, tag="o", name=f"oT{g}")
              for g in range(3)]
        pend = []

        def flush_av():
            while pend:
                kt, p01, p2 = pend.pop()
                av_step(oT[0], kt, p01[:, 0, :], kt == 0, kt == KT - 1)
                av_step(oT[1], kt, p01[:, 1, :], kt == 0, kt == KT - 1)
                av_step(oT[2], kt, p2, kt == 0, kt == KT - 1)

        def attention_kt(kt):
            X = spool.tile([128, 2, GROUP], f32, tag="s", name="X")
            s_step(kt, 0, X[:, 0, :])
            s_step(kt, 1, X[:, 1, :])
            p01 = pt01.tile([128, 2, GROUP], bf16, tag="p01", name="p01")
            nc.scalar.activation(p01, X, EXP)
            Y = spool.tile([128, 2, GROUP], f32, tag="s", name="Y")
            s_step(kt, 2, Y[:, 0, :])
            p2 = pt2.tile([128, GROUP], bf16, tag="p2", name="p2")
            nc.scalar.activation(p2, Y[:, 0, :], EXP)
            flush_av()
            pend.append((kt, p01, p2))

        proj_qk(xk_cur, wk_sb, bk_sb, kT, 0)
        proj_va_mm(xv_cur, 0)
        proj_qk(xq1, wq_sb, bq_sb, qT, 1)
        proj_qk(xq2, wq_sb, bq_sb, qT, 2)
        for kt in range(4):
            va_tp(kt)
        xk_next = load_x(x_k, 1)
        xv_next = load_x(x_v, 1)
        for g8 in range(1, KG):
            xk_cur, xv_cur = xk_next, xv_next
            if g8 < KG - 1:
                xk_next = load_x(x_k, g8 + 1)
                xv_next = load_x(x_v, g8 + 1)
            base = 4 * (g8 - 1)
            proj_qk(xk_cur, wk_sb, bk_sb, kT, g8)
            attention_kt(base)
            proj_va_mm(xv_cur, g8)
            attention_kt(base + 1)
            va_tp(4 * g8)
            va_tp(4 * g8 + 1)
            attention_kt(base + 2)
            va_tp(4 * g8 + 2)
            va_tp(4 * g8 + 3)
            if g8 == 1:
                # group-3 q projection, off the critical path
                proj_qk(load_x(x_q, 3), wq_sb, bq_sb, qT, 3)
            attention_kt(base + 3)
        for kt in range(4 * (KG - 1), 4 * KG):
            attention_kt(kt)
        flush_av()
        for g in range(3):
            epilogue(g, oT[g])

        # ---- pass 2: group 3 over resident kT/qT/va, kt-paired exps ----
        oT3 = opool.tile([MA, GROUP], f32, tag="o", name="oT3")
        pend3 = []
        for kp in range(KT // 2):
            Z = spool.tile([128, 2, GROUP], f32, tag="s", name="Z")
            s_step(2 * kp, 3, Z[:, 0, :])
            s_step(2 * kp + 1, 3, Z[:, 1, :])
            p3 = pt01.tile([128, 2, GROUP], bf16, tag="p01", name="p3")
            nc.scalar.activation(p3, Z, EXP)
            while pend3:
                pkp, pp = pend3.pop()
                av_step(oT3, 2 * pkp, pp[:, 0, :], pkp == 0, False)
                av_step(oT3, 2 * pkp + 1, pp[:, 1, :], False,
                        pkp == KT // 2 - 1)
            pend3.append((kp, p3))
        while pend3:
            pkp, pp = pend3.pop()
            av_step(oT3, 2 * pkp, pp[:, 0, :], pkp == 0, False)
            av_step(oT3, 2 * pkp + 1, pp[:, 1, :], False, pkp == KT // 2 - 1)
        epilogue(3, oT3)

        if debug:
            nc.sync.dma_start(out=dbg_qT[:], in_=qT)
            nc.sync.dma_start(out=dbg_kT[:], in_=kT)
            nc.sync.dma_start(out=dbg_va[:], in_=va)

    nc.finalize()
    return nc


def get_nc():
    if "nc" not in _CACHE:
        _CACHE["nc"] = _build()
    return _CACHE["nc"]


def make_in_maps(queries, keys, values, Wq, bq, Wk, bk, Wv, bv):
    bf = ml_dtypes.bfloat16

    def xt(a):  # [seq, D] fp32 -> transposed bf16 [D, seq]
        return np.ascontiguousarray(np.asarray(a, dtype=np.float32).T.astype(bf))

    def wpack(w, scale=1.0):  # [D, E] -> [128, CH, E] bf16
        w = np.asarray(w, dtype=np.float32) * scale
        return np.ascontiguousarray(
            w.reshape(CH, 128, E).transpose(1, 0, 2).astype(bf))

    queries = np.asarray(queries, dtype=np.float32)
    keys = np.asarray(keys, dtype=np.float32)
    values = np.asarray(values, dtype=np.float32)
    shared = {
        "w_q": wpack(Wq, SCALE), "w_k": wpack(Wk), "w_v": wpack(Wv),
        "b_q": np.ascontiguousarray(
            (np.asarray(bq, np.float32) * SCALE).reshape(E, 1)),
        "b_k": np.ascontiguousarray(np.asarray(bk, np.float32).reshape(E, 1)),
        "b_v": np.ascontiguousarray(np.asarray(bv, np.float32).reshape(E, 1)),
    }
    in_maps = []
    for c in range(NCORES):
        b, h = divmod(c, 2)
        in_maps.append({
            "x_q": xt(queries[b, h * HALF:(h + 1) * HALF, :]),
            "x_k": xt(keys[b]),
            "x_v": xt(values[b]),
            **shared,
        })
    return in_maps


def run(trace=False, **inputs):
    from concourse.bass_utils import run_bass_kernel_spmd

    nc = get_nc()
    in_maps = make_in_maps(**inputs)
    res = run_bass_kernel_spmd(
        nc, in_maps, core_ids=list(range(NCORES)), trace=trace)
    full = np.empty((B, N, E), dtype=np.float32)
    for c in range(NCORES):
        b, h = divmod(c, 2)
        o = np.asarray(res.results[c]["out"], dtype=np.float32)  # [66, 2048]
        full[b, h * HALF:(h + 1) * HALF, :] = (o[:E] / o[E:E + 1]).T
    return full, res


def kernel(**inputs):
    full, _ = run(trace=False, **inputs)
    return full


# revision 32
# speedup vs baseline: 1.1602x; 1.1602x over previous
"""Trainium2 Bass kernel for batched scaled-dot-product attention.

Problem (reference math in fp32):
    q = queries @ Wq + bq          [B=4, N=4096, E=64]   (D_MODEL=768)
    k = keys    @ Wk + bk
    v = values  @ Wv + bv
    out = softmax(q k^T / sqrt(E)) @ v                    [B, N, 64]

Sharding: 8 cores, data-parallel over batch x query-half.  Core c handles
batch b=c//2, query rows [h*2048, (h+1)*2048) with h=c%2; it loads the full
keys/values for its batch (softmax needs every key).

v2 design (vs the fp32r v1 baseline at ~176us):
  * Everything on the input path is bf16 (host-cast): x DMA bytes halve to
    ~12MB/core and every matmul runs at 1 cycle/row at any PE p-state.
    Verified numerically: end-to-end rel err ~5.5e-3 vs the 2e-2 gate.
  * No q/k row-doubling: bf16 matmuls don't need a 128-deep contraction to
    hit full rate (the moving-row stream is the limit either way).
  * The 1/sqrt(E) scale is folded into Wq/bq on the host.
  * v is projected straight into natural [seq,64] layout ("va-direct"):
    per 128-row tile, 6 matmuls with the x_v^T chunk as the stationary
    operand.  No PE/DMA transposes anywhere in the main pipeline.  Two ones
    columns are appended (va width 66) so attention row-sums fall out of
    the av matmul; normalization happens on the HOST after gather.
  * Attention in S^T layout.  Query groups 0-2 stream inline with the k/v
    projection (per k-tile: 3 S matmuls, a paired exp on groups 0+1 plus a
    single exp on group 2, 3 av accumulations).  Group 3 runs as a second
    pass over resident kT/qT/va with kt-paired exps.  This 3+1 split is
    what fits 8 PSUM banks: S pool 2x[128,2,512] (4) + oT 3x[66,512] (3) +
    projection accumulator (1).
  * exp is the ACT-engine floor (~55us of pure column throughput); pairing
    two 512-col scores tiles per activation instruction halves the ~143ns
    per-instruction overhead.  ACT does nothing but exp.
  * Output is written as oT [66, 2048] fp32 (64 value rows + rowsum row);
    the host does out = (oT[:64]/oT[64]).T -- no device epilogue transpose.
"""

import numpy as np
import ml_dtypes

B, N, D, E = 4, 4096, 768, 64
NCORES = 8
HALF = N // 2          # query rows per core
CH = D // 128          # 6 feature chunks of the contraction dim
GROUP = 512            # query columns per group
QG = HALF // GROUP     # 4 query groups per core
KT = N // 128          # 32 key tiles
KG = N // GROUP        # 8 k/v projection groups
MA = E + 2             # va width: 64 values + 2 ones columns (rowsum)
SCALE = 0.125          # 1/sqrt(E), folded into Wq/bq on the host

_CACHE = {}


def _build():
    from contextlib import ExitStack

    import concourse.mybir as mybir
    import concourse.tile as tile
    from concourse import bacc

    f32 = mybir.dt.float32
    bf16 = mybir.dt.bfloat16
    EXP = mybir.ActivationFunctionType.Exp

    nc = bacc.Bacc(trn_type="TRN2")
    x_q = nc.dram_tensor("x_q", [D, HALF], bf16, kind="ExternalInput")
    x_k = nc.dram_tensor("x_k", [D, N], bf16, kind="ExternalInput")
    x_v = nc.dram_tensor("x_v", [D, N], bf16, kind="ExternalInput")
    w_q = nc.dram_tensor("w_q", [128, CH, E], bf16, kind="ExternalInput")
    w_k = nc.dram_tensor("w_k", [128, CH, E], bf16, kind="ExternalInput")
    w_v = nc.dram_tensor("w_v", [128, CH, E], bf16, kind="ExternalInput")
    b_q = nc.dram_tensor("b_q", [E, 1], f32, kind="ExternalInput")
    b_k = nc.dram_tensor("b_k", [E, 1], f32, kind="ExternalInput")
    b_v4 = nc.dram_tensor("b_v4", [128, 4, E], bf16, kind="ExternalInput")
    out = nc.dram_tensor("out", [MA, HALF], f32, kind="ExternalOutput")
    import os
    debug = bool(os.environ.get("KERNEL_DEBUG_DUMP"))
    if debug:
        dbg_qT = nc.dram_tensor("dbg_qT", [E, HALF], bf16, kind="ExternalOutput")
        dbg_kT = nc.dram_tensor("dbg_kT", [E, N], bf16, kind="ExternalOutput")
        dbg_va = nc.dram_tensor("dbg_va", [128, KT, MA], bf16, kind="ExternalOutput")

    with tile.TileContext(nc) as tc, ExitStack() as ctx:
        singles = ctx.enter_context(tc.tile_pool(name="singles", bufs=1))
        wq_sb = singles.tile([128, CH, E], bf16)
        wk_sb = singles.tile([128, CH, E], bf16)
        wv_sb = singles.tile([128, CH, E], bf16)
        bq_sb = singles.tile([E, 1], f32)
        bk_sb = singles.tile([E, 1], f32)
        bv4_sb = singles.tile([128, 4, E], bf16)
        for dst, src in ((wq_sb, w_q), (wk_sb, w_k), (wv_sb, w_v),
                         (bq_sb, b_q), (bk_sb, b_k), (bv4_sb, b_v4)):
            nc.sync.dma_start(out=dst, in_=src[:])

        qT = singles.tile([E, HALF], bf16)      # q^T/8 (+bias)
        kT = singles.tile([E, N], bf16)         # k^T
        va = singles.tile([128, KT, MA], bf16)  # v natural + two ones columns
        nc.vector.memset(va[:, :, E:MA], 1.0)

        xpool = ctx.enter_context(tc.tile_pool(name="xT", bufs=8))
        pt01 = ctx.enter_context(tc.tile_pool(name="pt01", bufs=7))
        eppool = ctx.enter_context(tc.tile_pool(name="ep", bufs=2))
        spool = ctx.enter_context(tc.tile_pool(name="s", bufs=2, space="PSUM"))
        opool = ctx.enter_context(tc.tile_pool(name="o", bufs=3, space="PSUM"))
        pjpool = ctx.enter_context(tc.tile_pool(name="pj", bufs=1, space="PSUM"))

        def load_x(x_dr, g):
            xt = xpool.tile([128, CH, GROUP], bf16, tag="xT", name="xt")
            nc.sync.dma_start(
                out=xt,
                in_=x_dr[:, g * GROUP:(g + 1) * GROUP].rearrange(
                    "(c p) s -> p c s", p=128))
            return xt

        def proj_qk(xt, w_sb, b_sb, dst, g):
            ps = pjpool.tile([128, GROUP], f32, tag="pj", name="ps")
            for c in range(CH):
                nc.tensor.matmul(
                    ps[:E], lhsT=w_sb[:, c, :], rhs=xt[:, c, :],
                    start=(c == 0), stop=(c == CH - 1))
            nc.vector.tensor_scalar_add(
                dst[:, g * GROUP:(g + 1) * GROUP], ps[:E], b_sb)

        def proj_va(xt, g8):
            # va-direct: project straight into natural [seq,64] layout with
            # the x_v^T chunk as the stationary operand (PE transposes and
            # XBAR DMA transposes both measure far slower on hardware).
            # c must be the inner loop: a start=True resets the whole PSUM
            # bank's accumulation state, so groups cannot interleave.
            vj = pjpool.tile([128, 4, E], f32, tag="pj", name="vj")
            for t in range(4):
                for c in range(CH):
                    nc.tensor.matmul(
                        vj[:, t, :], lhsT=xt[:, c, t * 128:(t + 1) * 128],
                        rhs=wv_sb[:, c, :], start=(c == 0), stop=(c == CH - 1),
                        skip_group_check=True)
            nc.vector.tensor_add(va[:, g8 * 4:(g8 + 1) * 4, :E], vj, bv4_sb)

        def s_step(kt, g, out_ap):
            nc.tensor.matmul(
                out_ap,
                lhsT=kT[:, kt * 128:(kt + 1) * 128],
                rhs=qT[:, g * GROUP:(g + 1) * GROUP],
                start=True, stop=True, skip_group_check=True)

        def av_step(oT_g, kt, pt_ap, first, last):
            nc.tensor.matmul(
                oT_g, lhsT=va[:, kt, :], rhs=pt_ap,
                start=first, stop=last, skip_group_check=True)

        def epilogue(g, oT_g):
            o_sb = eppool.tile([MA, GROUP], f32, tag="ep", name="o_sb")
            nc.vector.tensor_copy(o_sb, oT_g)
            nc.sync.dma_start(out=out[:, g * GROUP:(g + 1) * GROUP], in_=o_sb)

        # ---- prologue: q0 + k/v group 0 first to minimize attention lag ----
        xq0 = load_x(x_q, 0)
        xk_cur = load_x(x_k, 0)
        xv_cur = load_x(x_v, 0)
        xq1 = load_x(x_q, 1)
        xq2 = load_x(x_q, 2)
        proj_qk(xq0, wq_sb, bq_sb, qT, 0)

        # ---- pass 1: k/v stream + attention for groups 0,1,2 ----
        # Software-pipelined: per kt we issue S matmuls + exps for kt but the
        # av accumulations for kt-1, so the in-order PE never waits on the
        # exp it just requested.  Attention kt blocks for group g8-1 are
        # issued around g8's projections so the PE has work while the DVE
        # drains the single projection PSUM bank.
        oT = [opool.tile([MA, GROUP], f32, tag="o", name=f"oT{g}")
              for g in range(3)]
        pend = []

        def flush_av():
            while pend:
                kt, p01, p2ap = pend.pop(0)  # FIFO: kt==0 start must go first
                av_step(oT[0], kt, p01[:, 0, :], kt == 0, kt == KT - 1)
                av_step(oT[1], kt, p01[:, 1, :], kt == 0, kt == KT - 1)
                av_step(oT[2], kt, p2ap, kt == 0, kt == KT - 1)

        def attention_pair(kt):
            # Covers kt and kt+1.  The group-2 scores of both kts share one
            # paired exp so every ACTIVATE is N=1024 (amortizes the 352-cycle
            # fixed cost per instruction).
            p01s = []
            for k in (kt, kt + 1):
                X = spool.tile([128, 2, GROUP], f32, tag="s", name="X")
                s_step(k, 0, X[:, 0, :])
                s_step(k, 1, X[:, 1, :])
                p01 = pt01.tile([128, 2, GROUP], bf16, tag="pt", name="p01")
                nc.scalar.activation(p01, X, EXP)
                p01s.append(p01)
            Y = spool.tile([128, 2, GROUP], f32, tag="s", name="Y")
            s_step(kt, 2, Y[:, 0, :])
            s_step(kt + 1, 2, Y[:, 1, :])
            p2 = pt01.tile([128, 2, GROUP], bf16, tag="pt", name="p2")
            nc.scalar.activation(p2, Y, EXP)
            flush_av()
            for j, k in enumerate((kt, kt + 1)):
                pend.append((k, p01s[j], p2[:, j, :]))

        proj_qk(xk_cur, wk_sb, bk_sb, kT, 0)
        proj_qk(xq1, wq_sb, bq_sb, qT, 1)
        proj_qk(xq2, wq_sb, bq_sb, qT, 2)
        proj_va(xv_cur, 0)
        xk_next = load_x(x_k, 1)
        xv_next = load_x(x_v, 1)
        for g8 in range(1, KG):
            xk_cur, xv_cur = xk_next, xv_next
            if g8 < KG - 1:
                xk_next = load_x(x_k, g8 + 1)
                xv_next = load_x(x_v, g8 + 1)
            base = 4 * (g8 - 1)
            proj_qk(xk_cur, wk_sb, bk_sb, kT, g8)
            attention_pair(base)
            proj_va(xv_cur, g8)
            if g8 == 1:
                # group-3 q projection, off the critical path
                proj_qk(load_x(x_q, 3), wq_sb, bq_sb, qT, 3)
            attention_pair(base + 2)
        attention_pair(4 * (KG - 1))
        attention_pair(4 * (KG - 1) + 2)
        flush_av()
        for g in range(3):
            epilogue(g, oT[g])

        # ---- pass 2: group 3 over resident kT/qT/va, kt-paired exps ----
        oT3 = opool.tile([MA, GROUP], f32, tag="o", name="oT3")
        pend3 = []
        for kp in range(KT // 2):
            Z = spool.tile([128, 2, GROUP], f32, tag="s", name="Z")
            s_step(2 * kp, 3, Z[:, 0, :])
            s_step(2 * kp + 1, 3, Z[:, 1, :])
            p3 = pt01.tile([128, 2, GROUP], bf16, tag="pt", name="p3")
            nc.scalar.activation(p3, Z, EXP)
            while pend3:
                pkp, pp = pend3.pop()
                av_step(oT3, 2 * pkp, pp[:, 0, :], pkp == 0, False)
                av_step(oT3, 2 * pkp + 1, pp[:, 1, :], False,
                        pkp == KT // 2 - 1)
            pend3.append((kp, p3))
        while pend3:
            pkp, pp = pend3.pop()
            av_step(oT3, 2 * pkp, pp[:, 0, :], pkp == 0, False)
            av_step(oT3, 2 * pkp + 1, pp[:, 1, :], False, pkp == KT // 2 - 1)
        epilogue(3, oT3)

        if debug:
            nc.sync.dma_start(out=dbg_qT[:], in_=qT)
            nc.sync.dma_start(out=dbg_kT[:], in_=kT)
            nc.sync.dma_start(out=dbg_va[:], in_=va)

    nc.finalize()
    return nc


def get_nc():
    if "nc" not in _CACHE:
        _CACHE["nc"] = _build()
    return _CACHE["nc"]


def make_in_maps(queries, keys, values, Wq, bq, Wk, bk, Wv, bv):
    bf = ml_dtypes.bfloat16

    def xt(a):  # [seq, D] fp32 -> transposed bf16 [D, seq]
        return np.ascontiguousarray(np.asarray(a, dtype=np.float32).T.astype(bf))

    def wpack(w, scale=1.0):  # [D, E] -> [128, CH, E] bf16
        w = np.asarray(w, dtype=np.float32) * scale
        return np.ascontiguousarray(
            w.reshape(CH, 128, E).transpose(1, 0, 2).astype(bf))

    queries = np.asarray(queries, dtype=np.float32)
    keys = np.asarray(keys, dtype=np.float32)
    values = np.asarray(values, dtype=np.float32)
    shared = {
        "w_q": wpack(Wq, SCALE), "w_k": wpack(Wk), "w_v": wpack(Wv),
        "b_q": np.ascontiguousarray(
            (np.asarray(bq, np.float32) * SCALE).reshape(E, 1)),
        "b_k": np.ascontiguousarray(np.asarray(bk, np.float32).reshape(E, 1)),
        "b_v4": np.ascontiguousarray(np.broadcast_to(
            np.asarray(bv, np.float32).astype(bf), (128, 4, E))),
    }
    in_maps = []
    for c in range(NCORES):
        b, h = divmod(c, 2)
        in_maps.append({
            "x_q": xt(queries[b, h * HALF:(h + 1) * HALF, :]),
            "x_k": xt(keys[b]),
            "x_v": xt(values[b]),
            **shared,
        })
    return in_maps


def run(trace=False, **inputs):
    from concourse.bass_utils import run_bass_kernel_spmd

    nc = get_nc()
    in_maps = make_in_maps(**inputs)
    res = run_bass_kernel_spmd(
        nc, in_maps, core_ids=list(range(NCORES)), trace=trace)
    full = np.empty((B, N, E), dtype=np.float32)
    for c in range(NCORES):
        b, h = divmod(c, 2)
        o = np.asarray(res.results[c]["out"], dtype=np.float32)  # [66, 2048]
        full[b, h * HALF:(h + 1) * HALF, :] = (o[:E] / o[E:E + 1]).T
    return full, res


def kernel(**inputs):
    full, _ = run(trace=False, **inputs)
    return full
